# revision 1
# baseline (speedup 1.0000x reference)
"""Causal self-attention with RoPE for Trainium2, 8-way SPMD.

Sharding: data-parallel over batch (2) x tensor-parallel over head-groups (4).
Core c handles batch c//4, heads 4*(c%4) .. 4*(c%4)+3.  Each core computes
q/k/v projections for its head columns of W_qkv, attention for its 4 heads,
and a partial output projection against its rows of W_proj.  The all-reduce
over the 4 cores of each batch plus b_proj happens on the host.

Device layout per core:
  xT    [D=1024, T=2048]   x transposed (contraction on partitions)
  Q^T/K^T stored [128, 2, T]: tile hp holds heads {2hp, 2hp+1} (64 rows each)
  V     stored [128, NT, 4*65]: per t-block, per head 64 value cols + ones col
        (the ones column makes the PV matmul emit the softmax denominator)
  S^T   computed per (head, k-block) as [k=128, q<=512] so softmax's sum over
        k falls out of the PV matmul; exp runs on ACT with scale=1/8 fused.

All matmul operands are float32r (fp32 bytes, PE rounds the multiplicands) —
4x faster than strict fp32 on the PE.  Production of K/V/Q is interleaved
with per-q-tile attention so ACT (exp) and PE (matmul) overlap end to end.
"""
import sys
sys.path.insert(0, "/opt/trn_rl_repo")

from contextlib import ExitStack

import ml_dtypes
import numpy as np

import concourse.bass as bass
import concourse.tile as tile
from concourse import bacc, mybir

B, T, D, H, HD = 2, 2048, 1024, 16, 64
NCORES = 8
GROUPS = 4            # tensor-parallel head groups
HPG = H // GROUPS     # heads per core
M = HPG * HD          # per-core projection width (256)
NT = T // 128         # 16 t-blocks
NQT = T // 512        # 4 q-tiles
NKT = D // 128        # 8 contraction tiles for the qkv projection
F32 = mybir.dt.float32
F32R = mybir.dt.float32r
AF = mybir.ActivationFunctionType


def _rope_tables():
    # mirrors reference._rope_cache in float32
    inv = (1.0 / (10000.0 ** (np.arange(0, HD, 2, dtype=np.float32) / HD))).astype(np.float32)
    t = np.arange(T, dtype=np.float32)
    fr = t[:, None] * inv[None, :]                    # [T, 32]
    cos32 = np.cos(fr).T.astype(np.float32)           # [32, T]
    sin32 = np.sin(fr).T.astype(np.float32)
    c2 = np.tile(cos32, (4, 1))                                        # [128, T]
    s2 = np.concatenate([-sin32, sin32, -sin32, sin32], 0)             # [128, T]
    return c2, s2


def _mask_plan(mask):
    """Classify 128x128 blocks of mask^T and build the per-(q-tile, k-block)
    schedule: (lo, hi, [(col_off, mix_id), ...]) with lo/hi relative to the
    512-wide q-tile, or None when the whole block is masked out.  mix tiles
    are the partially-masked 128x128 blocks of mask^T (0/1 f32)."""
    mt = (np.asarray(mask).T != 0)
    nb = T // 128
    state = np.empty((nb, nb), np.int8)               # [k-block, q-block]
    for ki in range(nb):
        for qi in range(nb):
            sub = mt[128 * ki:128 * (ki + 1), 128 * qi:128 * (qi + 1)]
            state[ki, qi] = 2 if sub.all() else (1 if sub.any() else 0)
    mix_tiles = []
    mix_idx = {}
    zero_id = None

    def _mix_id(ki, qi):
        key = (ki, qi)
        if key not in mix_idx:
            mix_idx[key] = len(mix_tiles)
            mix_tiles.append(
                mt[128 * ki:128 * (ki + 1), 128 * qi:128 * (qi + 1)].astype(np.float32))
        return mix_idx[key]

    plan = []
    for j in range(NQT):
        row = []
        for kb in range(nb):
            sts = [state[kb, 4 * j + q] for q in range(4)]
            nz = [q for q in range(4) if sts[q] != 0]
            if not nz:
                row.append(None)
                continue
            lo_q, hi_q = nz[0], nz[-1] + 1
            mixes = []
            for q in range(lo_q, hi_q):
                if sts[q] == 1:
                    mixes.append((128 * q, _mix_id(kb, 4 * j + q)))
                elif sts[q] == 0:
                    # hole inside the window: mask it to zero
                    if zero_id is None:
                        mix_idx[("zero",)] = len(mix_tiles)
                        mix_tiles.append(np.zeros((128, 128), np.float32))
                        zero_id = mix_idx[("zero",)]
                    mixes.append((128 * q, zero_id))
            row.append((128 * lo_q, 128 * hi_q, mixes))
        plan.append(row)
    if mix_tiles:
        mm = np.concatenate(mix_tiles, axis=1)        # [128, n_mix*128]
    else:
        mm = np.zeros((128, 128), np.float32)
    return plan, mm


def build_program(plan, n_mix_cols, dbg=False, variant=(), zero_bias=False):
    nc = bacc.Bacc("TRN2", target_bir_lowering=False, debug=False,
                   num_devices=NCORES)
    dram = {}
    for name, shape, dt in [
        ("xT", [D, T], F32R), ("wq", [D, M], F32R), ("wk", [D, M], F32R),
        ("wv", [D, M], F32R), ("wp", [M, D], F32R), ("bq", [M, 1], F32),
        ("bk", [M, 1], F32), ("bvb", [128, M], F32),
        ("c2", [128, T], F32), ("s2", [128, T], F32),
        ("mm", [128, n_mix_cols], mybir.dt.bfloat16), ("vones", [128, NT * HPG], F32R),
    ]:
        dram[name] = nc.dram_tensor(name, shape, dt, kind="ExternalInput").ap()
    y = nc.dram_tensor("y", [T, D], F32, kind="ExternalOutput").ap()
    dbg_t = {}
    if dbg:
        for name, shape, dt in [
            ("dqT", [128, 2 * T], F32R), ("dkT", [128, 2 * T], F32R),
            ("dv", [128, NT * HPG * 65], F32R), ("dpt", [128, 4 * 1024], F32R),
            ("dyt", [128, 2 * 512], F32R), ("drb", [64, 2 * 512], F32),
            ("drr", [1, 2 * 512], F32),
        ]:
            dbg_t[name] = nc.dram_tensor(name, shape, dt,
                                         kind="ExternalOutput").ap()

    with tile.TileContext(nc) as tc, ExitStack() as ctx:
        persist = ctx.enter_context(tc.tile_pool(name="persist", bufs=1))
        qT = persist.tile([128, 2, T], F32R, name="qT", tag="qT")
        kT = persist.tile([128, 2, T], F32R, name="kT", tag="kT")
        v_sb = persist.tile([128, NT, HPG * 65], F32R, name="v", tag="v")
        c2_sb = persist.tile([128, T], F32, name="c2", tag="c2")
        s2_sb = persist.tile([128, T], F32, name="s2", tag="s2")
        bqk_sb = persist.tile([128, 4], F32, name="bqk", tag="bqk")
        bvb_sb = persist.tile([128, M], F32, name="bvb", tag="bvb")
        wp_sb = persist.tile([128, 2, D], F32R, name="wp", tag="wp")
        mm_sb = persist.tile([128, n_mix_cols], mybir.dt.bfloat16, name="mm", tag="mm")

        pw = ctx.enter_context(tc.tile_pool(name="w", bufs=1))
        px = ctx.enter_context(tc.tile_pool(name="x", bufs=1))
        praw = ctx.enter_context(tc.tile_pool(name="praw", bufs=2))
        prt = ctx.enter_context(tc.tile_pool(name="ropetmp", bufs=1))
        pt_pool = ctx.enter_context(tc.tile_pool(name="pt", bufs=3))
        yt_pool = ctx.enter_context(tc.tile_pool(name="yt", bufs=2))
        ro_pool = ctx.enter_context(tc.tile_pool(name="ro", bufs=1))
        rb_pool = ctx.enter_context(tc.tile_pool(name="rb", bufs=1))
        out_pool = ctx.enter_context(tc.tile_pool(name="out", bufs=3))
        psA = ctx.enter_context(tc.tile_pool(name="psA", bufs=1, space="PSUM"))
        psS = ctx.enter_context(tc.tile_pool(name="psS", bufs=3, space="PSUM"))
        psPV = ctx.enter_context(tc.tile_pool(name="psPV", bufs=4, space="PSUM"))

        # DMA order = first-use order: wk, xT (first K chain), then wv/wq
        w_sbs = {}
        for wname in ("wk", "wv", "wq"):
            w_sbs[wname] = pw.tile([128, NKT, M], F32R, name=wname, tag=wname)
        nc.sync.dma_start(
            w_sbs["wk"], dram["wk"].rearrange("(kt p) m -> p kt m", p=128))
        xT_sb = px.tile([128, NKT, T], F32R, name="xT", tag="xT")
        for kt in range(NKT):
            nc.sync.dma_start(xT_sb[:, kt, :],
                              dram["xT"][128 * kt:128 * (kt + 1), :])
        for wname in ("wv", "wq"):
            nc.sync.dma_start(
                w_sbs[wname], dram[wname].rearrange("(kt p) m -> p kt m", p=128))
        nc.sync.dma_start(c2_sb, dram["c2"])
        nc.sync.dma_start(s2_sb, dram["s2"])
        nc.sync.dma_start(bqk_sb[:, 0:1], dram["bq"][0:128])
        nc.sync.dma_start(bqk_sb[:, 1:2], dram["bq"][128:256])
        nc.sync.dma_start(bqk_sb[:, 2:3], dram["bk"][0:128])
        nc.sync.dma_start(bqk_sb[:, 3:4], dram["bk"][128:256])
        nc.sync.dma_start(bvb_sb, dram["bvb"])
        nc.sync.dma_start(wp_sb, dram["wp"].rearrange("(kt p) m -> p kt m", p=128))
        nc.sync.dma_start(mm_sb, dram["mm"])
        # ones columns of V (the value copies only overwrite cols 0..63/head);
        # memset can't encode f32r, so DMA the ones in from DRAM
        nc.sync.dma_start(
            v_sb.rearrange("p a (h e) -> p a h e", e=65)[:, :, :, 64:65],
            dram["vones"].rearrange("p (a h) -> p a h", h=HPG)[:, :, :, None])

        def qk_chunk(w_sb, bias_col, dst, jt, tt):
            """produce rope'd projection chunk dst[:, jt, 512tt:512tt+512]."""
            cs = slice(512 * tt, 512 * (tt + 1))
            ps = psA.tile([128, 512], F32, name="psA", tag="psA")
            for kt in range(NKT):
                nc.tensor.matmul(ps, w_sb[:, kt, 128 * jt:128 * (jt + 1)],
                                 xT_sb[:, kt, cs],
                                 start=(kt == 0), stop=(kt == NKT - 1))
            raw = praw.tile([128, 512], F32, name="raw", tag="raw")
            if zero_bias:
                nc.vector.tensor_copy(raw, ps)
            else:
                nc.scalar.activation(raw, ps, AF.Identity,
                                     bias=bqk_sb[:, bias_col:bias_col + 1])
            # rope: dst = raw*c2 + swap32(raw)*s2 (sign baked into s2).
            # the 32-row partition swap must ride a DMA (engines cannot
            # shift partitions); issue from ACT's queue to spare SP.
            rsw = praw.tile([128, 512], F32, name="rsw", tag="rsw")
            for h2 in range(2):
                for half in range(2):
                    d0 = 64 * h2 + 32 * half
                    s0 = 64 * h2 + 32 * (1 - half)
                    nc.scalar.dma_start(rsw[d0:d0 + 32, :], raw[s0:s0 + 32, :])
            t2 = prt.tile([128, 512], F32, name="t2", tag="t2")
            dstc = dst[:, jt, cs]
            nc.vector.tensor_mul(t2, rsw, s2_sb[:, cs])
            nc.vector.tensor_mul(dstc, raw, c2_sb[:, cs])
            nc.vector.tensor_add(dstc, dstc.bitcast(F32), t2)

        def v_block(tb):
            ps = psA.tile([128, 512], F32, name="psV", tag="psA")
            for kt in range(NKT):
                nc.tensor.matmul(ps[:, 0:256],
                                 xT_sb[:, kt, 128 * tb:128 * (tb + 1)],
                                 w_sbs["wv"][:, kt, :],
                                 start=(kt == 0), stop=(kt == NKT - 1))
            vdst = v_sb[:, tb, :].rearrange("p (h e) -> p h e", e=65)[:, :, 0:64]
            psv = ps[:, 0:256].rearrange("p (h d) -> p h d", d=64)
            if zero_bias:
                nc.vector.tensor_copy(vdst, psv)
            else:
                nc.vector.tensor_add(vdst, psv,
                                     bvb_sb.rearrange("p (h d) -> p h d", d=64))

        pv_live = {}

        def attention_scores(j):
            yts_pv = []
            for hp in range(2):
                pvs = [psPV.tile([128, 512], F32, name="pv", tag="pv")
                       for _ in range(2)]
                kbs = [kb for kb in range(NT) if plan[j][kb] is not None]
                started = [False, False]
                def emit_pv(kb, pts):
                    lo, hi, _ = plan[j][kb]
                    for h2 in range(2):
                        gh = 2 * hp + h2
                        nc.tensor.matmul(
                            pvs[h2][0:65, lo:hi],
                            v_sb[:, kb, 65 * gh:65 * gh + 65],
                            pts[h2][:, lo:hi],
                            start=(not started[h2]),
                            stop=(kb == kbs[-1]))
                        started[h2] = True

                pending = None            # (kb, pts) with exp done, PV not yet
                for kb in kbs:
                    lo, hi, mixes = plan[j][kb]
                    # widen the matmul window to >=256 cols: f32r drops to
                    # 1/4 rate below 256. exp/PV still use the exact window.
                    wlo = min(lo, max(0, hi - 256))
                    sps = [psS.tile([128, 512], F32, name="sp", tag="s")
                           for _ in range(2)]
                    pts = [pt_pool.tile([128, 512], F32R, name="pt", tag="pt")
                           for _ in range(2)]
                    for h2 in range(2):
                        nc.tensor.matmul(
                            sps[h2][:, wlo:hi],
                            kT[64 * h2:64 * (h2 + 1), hp,
                               128 * kb:128 * (kb + 1)],
                            qT[64 * h2:64 * (h2 + 1), hp,
                               512 * j + wlo:512 * j + hi],
                            start=True, stop=True,
                            tile_position=(64 * h2, 0))
                    # PV for the previous k-block: its exp has had a full
                    # S^T slot to complete, so the in-order PE stream does
                    # not stall on ACT
                    if pending is not None:
                        emit_pv(*pending)
                    for h2 in range(2):
                        nc.scalar.activation(
                            pts[h2][:, lo:hi], sps[h2][:, lo:hi],
                            AF.Exp, scale=1.0 / np.sqrt(HD))
                        for coff, mid in mixes:
                            pslice = pts[h2][:, coff:coff + 128]
                            nc.vector.tensor_mul(
                                pslice, pslice.bitcast(F32),
                                mm_sb[:, 128 * mid:128 * (mid + 1)])
                        if dbg and j == 0 and hp == 0 and kb == kbs[0]:
                            nc.sync.dma_start(
                                dbg_t["dpt"][:, 2048 * h2:2048 * h2 + 512],
                                pts[h2][:, 0:512])
                    pending = (kb, pts)
                if pending is not None:
                    emit_pv(*pending)
                yts_pv.append(pvs)
            pv_live[j] = yts_pv

        def attention_finish(j):
            yts = []
            pvs_all = pv_live.pop(j)
            for hp in range(2):
                pvs = pvs_all[hp]
                yt = yt_pool.tile([128, 512], F32R, name="yt", tag="yt")
                for h2 in range(2):
                    # denominator sits on psum partition 64
                    r = ro_pool.tile([65, 512], F32, name="r", tag="r")
                    nc.vector.reciprocal(r[64:65, :], pvs[h2][64:65, :])
                    # partition_broadcast sources the tile's partition 0:
                    # DMA-shift the row down first
                    r0 = ro_pool.tile([1, 512], F32, name="r0", tag="r0")
                    nc.sync.dma_start(r0, r[64:65, :])
                    rb = rb_pool.tile([64, 512], F32, name="rb", tag="rb")
                    nc.gpsimd.partition_broadcast(rb, r0)
                    if dbg and j == 0 and hp == 0:
                        nc.sync.dma_start(
                            dbg_t["drr"][:, 512 * h2:512 * (h2 + 1)],
                            r[64:65, :])
                        nc.sync.dma_start(
                            dbg_t["drb"][:, 512 * h2:512 * (h2 + 1)], rb)
                    if h2 == 0:
                        nc.vector.tensor_mul(yt[0:64, :], pvs[h2][0:64, :], rb)
                    else:
                        # rows 64..127: compute at 0..63 then DMA-shift
                        tmp = rb_pool.tile([64, 512], F32R, name="tmp",
                                           tag="tmp")
                        nc.vector.tensor_mul(tmp, pvs[h2][0:64, :], rb)
                        nc.sync.dma_start(yt[64:128, :], tmp)
                if dbg and j == 0:
                    nc.sync.dma_start(
                        dbg_t["dyt"][:, 512 * hp:512 * (hp + 1)], yt)
                yts.append(yt)
            for tb in range(4):
                ob = out_pool.tile([128, D], F32, name="ob", tag="ob")
                for nn in range(2):
                    po = psPV.tile([128, 512], F32, name="po", tag="pv")
                    for kt2 in range(2):
                        nc.tensor.matmul(
                            po, yts[kt2][:, 128 * tb:128 * (tb + 1)],
                            wp_sb[:, kt2, 512 * nn:512 * (nn + 1)],
                            start=(kt2 == 0), stop=(kt2 == 1))
                    nc.any.tensor_copy(ob[:, 512 * nn:512 * (nn + 1)], po)
                nc.sync.dma_start(
                    y[512 * j + 128 * tb:512 * j + 128 * (tb + 1), :], ob)

        # interleaved emission. engines run their streams IN ORDER, so the
        # next t-chunk's matmul chains are emitted between a q-tile's PV
        # (scores) and its normalize+projection (finish): PE crunches the
        # next chains while DVE/ACT drain the normalize.
        def needed_chunk(j):
            kbs = [kb for kb in range(NT) if plan[j][kb] is not None]
            need = max(kb // 4 for kb in kbs) if kbs else 0
            return max(need, j)

        produced = -1

        def ensure_chunks(up_to):
            nonlocal produced
            for t in range(produced + 1, min(up_to, NQT - 1) + 1):
                qk_chunk(w_sbs["wk"], 2 + 0, kT, 0, t)
                qk_chunk(w_sbs["wk"], 2 + 1, kT, 1, t)
                for tb in range(4 * t, 4 * t + 4):
                    v_block(tb)
                qk_chunk(w_sbs["wq"], 0, qT, 0, t)
                qk_chunk(w_sbs["wq"], 1, qT, 1, t)
                produced = t

        if "nostageb" in variant:
            ensure_chunks(NQT - 1)
        else:
            for j in range(NQT):
                ensure_chunks(needed_chunk(j))
                attention_scores(j)
                if j + 1 < NQT:
                    ensure_chunks(j + 1)
                attention_finish(j)

        if dbg:
            nc.sync.dma_start(dbg_t["dqT"], qT.rearrange("p a t -> p (a t)"))
            nc.sync.dma_start(dbg_t["dkT"], kT.rearrange("p a t -> p (a t)"))
            nc.sync.dma_start(dbg_t["dv"], v_sb.rearrange("p a e -> p (a e)"))
    nc.finalize()
    return nc


def make_core_inputs(x, mask, W_qkv, b_qkv, W_proj, b_proj, mm):
    """Per-core input dicts (all float32 numpy)."""
    x = np.asarray(x, np.float32)
    W_qkv = np.asarray(W_qkv, np.float32)
    b_qkv = np.asarray(b_qkv, np.float32)
    W_proj = np.asarray(W_proj, np.float32)
    c2, s2 = _rope_tables()
    in_maps = []
    for c in range(NCORES):
        b, g = divmod(c, GROUPS)
        xT = np.ascontiguousarray(x[b].T)
        sl = slice(M * g, M * (g + 1))
        wq = np.ascontiguousarray(W_qkv[:, 0 * D:1 * D][:, sl])
        wk = np.ascontiguousarray(W_qkv[:, 1 * D:2 * D][:, sl])
        wv = np.ascontiguousarray(W_qkv[:, 2 * D:3 * D][:, sl])
        bq = np.ascontiguousarray(b_qkv[0 * D:1 * D][sl]).reshape(M, 1)
        bk = np.ascontiguousarray(b_qkv[1 * D:2 * D][sl]).reshape(M, 1)
        bv = np.ascontiguousarray(b_qkv[2 * D:3 * D][sl])
        bvb = np.tile(bv[None, :], (128, 1)).astype(np.float32)
        wp = np.ascontiguousarray(W_proj[sl, :])
        in_maps.append({
            "xT": xT, "wq": wq, "wk": wk, "wv": wv, "wp": wp,
            "bq": bq, "bk": bk, "bvb": bvb, "c2": c2, "s2": s2,
            "mm": mm.astype(ml_dtypes.bfloat16),
            "vones": np.ones((128, NT * HPG), np.float32),
        })
    return in_maps


def gather_output(results, b_proj):
    out = np.zeros((B, T, D), np.float32)
    for c in range(NCORES):
        b = c // GROUPS
        out[b] += results[c]["y"]
    out += np.asarray(b_proj, np.float32)[None, None, :]
    return out


def kernel(x, mask, W_qkv, b_qkv, W_proj, b_proj):
    from concourse.bass_utils import run_bass_kernel_spmd
    plan, mm = _mask_plan(mask)
    zb = not (np.any(np.asarray(b_qkv)))
    nc = build_program(plan, mm.shape[1], zero_bias=zb)
    in_maps = make_core_inputs(x, mask, W_qkv, b_qkv, W_proj, b_proj, mm)
    res = run_bass_kernel_spmd(nc, in_maps, list(range(NCORES)))
    return gather_output(res.results, b_proj)

